# revision 7
# baseline (speedup 1.0000x reference)
"""Trainium2 Bass kernel for GNN message passing (8 NeuronCores, SPMD).

Sharding: edges sorted by receiver, sharded across 8 cores at receiver
boundaries -> each core owns a disjoint receiver range, so node partials
need no cross-core reduction; only the tiny [19, G] graph partials are
AllReduced.

Device pipeline per core:
  perm_nodes L2..L5 (block-diag b=4) -> c1; 10x upd_edges (b=8, c1
  re-added via identity matmul); segment-sum via PE-transpose + one-hot
  matmuls over static sliding 128-node windows (stride 56), evacuated to
  DRAM through a sliding accumulator; node-side MLPs (same machinery);
  per-graph one-hot segment sum; AllReduce; predict MLP.

SELU is exact: with E = exp(y + ln a),  selu(y) = l*(relu(y) + min(E-a, 0)).
The (t=relu, q=min) branches feed the next layer as K-split accumulate
matmuls sharing one lambda-scaled stationary.
"""

import math
import numpy as np
import ml_dtypes

SELU_L = 1.0507009873554805
SELU_A = 1.6732632423543772
LN_A = math.log(SELU_A)
BF16 = ml_dtypes.bfloat16

last_run_info = {}


def _np(x, dt=None):
    a = np.asarray(x)
    return a.astype(dt) if dt is not None else a


def selu_np(x):
    return SELU_L * np.where(x > 0, x, SELU_A * (np.exp(np.minimum(x, 0)) - 1.0))


def pack_cols(arr, b):
    """[E, w] -> [w*b, E/b]: out[w*j + f, c] = arr[b*c + j, f]."""
    E, w = arr.shape
    return np.ascontiguousarray(
        arr.reshape(E // b, b, w).transpose(1, 2, 0).reshape(b * w, E // b))


def blockdiag(W, b):
    i, o = W.shape
    out = np.zeros((i * b, o * b), dtype=np.float32)
    for k in range(b):
        out[k * i:(k + 1) * i, k * o:(k + 1) * o] = W
    return out


def rep_bias(bvec, b):
    return np.tile(np.asarray(bvec, np.float32), b).reshape(-1, 1)


def _pack_chunks(recv_rel, stride, cap, win):
    n = len(recv_rel)
    chunk_of = np.empty(n, np.int32)
    pos, ch = 0, 0
    while pos < n:
        base = stride * ch
        if recv_rel[pos] < base:
            return None
        hi_mand = np.searchsorted(recv_rel, base + stride, side="left")
        hi_opt = np.searchsorted(recv_rel, base + win, side="left")
        if hi_mand - pos > cap:
            return None
        take = max(min(hi_opt - pos, cap), hi_mand - pos)
        chunk_of[pos:pos + take] = ch
        pos += take
        ch += 1
    return chunk_of, ch


def kernel(**inputs):
    nodes = _np(inputs["nodes"], np.float32)
    edges = _np(inputs["edges"], np.float32)
    receivers = _np(inputs["receivers"], np.int64)
    senders = _np(inputs["senders"], np.int64)
    node_graph = _np(inputs["node_graph"], np.int64)
    G = int(np.asarray(inputs["num_graphs"]))
    params = inputs["params"]

    def lay(name):
        return [(np.asarray(W, np.float32), np.asarray(b, np.float32))
                for (W, b) in params[name]]

    pn, ue, pe, un, pr = (lay(k) for k in
                          ("perm_nodes", "upd_edges", "perm_edges",
                           "upd_nodes", "predict"))

    N, E = nodes.shape[0], edges.shape[0]
    NCORE = 8
    CAP, WIN, STRIDE, SUP = 1024, 128, 56, 16

    # ---- host: perm_nodes layer 1 (factorized gather + selu) --------------
    W1, b1 = pn[0]
    A = nodes @ W1[:3] + b1
    Bn = nodes @ W1[3:]
    s1_full = selu_np(A[receivers] + Bn[senders]).astype(BF16)

    # ---- shard by receiver range ------------------------------------------
    order = np.argsort(receivers, kind="stable")
    recv_s = receivers[order]
    cuts = [0]
    for i in range(1, NCORE):
        tgt = i * E // NCORE
        cuts.append(int(np.searchsorted(recv_s, recv_s[min(tgt, E - 1)],
                                        side="left")))
    cuts.append(E)
    r_lo = [int(recv_s[cuts[i]]) if cuts[i] < E else N for i in range(NCORE)]
    r_hi = r_lo[1:] + [N]

    per_core = []
    for c in range(NCORE):
        lo, hi = cuts[c], cuts[c + 1]
        rrel = (recv_s[lo:hi] - r_lo[c]).astype(np.int64)
        res = None
        for st in (STRIDE, 48, 40, 32):
            res = _pack_chunks(rrel, st, CAP, WIN)
            if res is not None:
                STRIDE = min(STRIDE, st)
                break
        assert res is not None
        per_core.append([order[lo:hi], rrel, res[0], res[1]])
    # re-pack all with the common (possibly reduced) stride
    for c in range(NCORE):
        res = _pack_chunks(per_core[c][1], STRIDE, CAP, WIN)
        assert res is not None
        per_core[c][2], per_core[c][3] = res

    NC_nodes = max(h - l for l, h in zip(r_lo, r_hi))
    NP = ((max(NC_nodes, 1) + 8191) // 8192) * 8192
    TCH = max(max(pc[3] for pc in per_core),
              (NP - WIN + STRIDE - 1) // STRIDE)
    TCH = ((TCH + SUP - 1) // SUP) * SUP
    EP = TCH * CAP
    NSUP = TCH // SUP

    # ---- per-core packed arrays -------------------------------------------
    s1p = np.zeros((NCORE, 64, EP // 4), BF16)
    e0p = np.zeros((NCORE, 24, EP // 8), BF16)
    rlocE = np.full((NCORE, 128, 8 * TCH), -1.0, BF16)
    n0p = np.zeros((NCORE, 24, NP // 8), BF16)
    rlocN8 = np.full((NCORE, 128, 8 * (NP // 1024)), -1.0, BF16)
    rlocN4 = np.full((NCORE, 128, 4 * (NP // 512)), -1.0, BF16)

    for c in range(NCORE):
        idx, rrel, chunk_of, tch = per_core[c]
        ne = len(idx)
        if ne:
            ch_starts = np.searchsorted(chunk_of, np.arange(chunk_of.max() + 1))
            pos = np.arange(ne) - ch_starts[chunk_of]
            slot = chunk_of.astype(np.int64) * CAP + pos
        else:
            slot = np.zeros(0, np.int64)

        se = np.zeros((EP, 16), BF16)
        se[slot] = s1_full[idx]
        s1p[c] = pack_cols(se, 4)

        ee = np.zeros((EP, 3), np.float32)
        ee[slot] = edges[idx]
        e0p[c] = pack_cols(ee.astype(BF16), 8)

        rl = np.full(EP, -1.0, np.float32)
        rl[slot] = (rrel - np.int64(STRIDE) * chunk_of).astype(np.float32)
        rlocE[c] = (rl.reshape(TCH, 128, 8).transpose(1, 0, 2)
                    .reshape(128, TCH * 8).astype(BF16))

        nc_n = r_hi[c] - r_lo[c]
        nn = np.zeros((NP, 3), np.float32)
        nn[:nc_n] = nodes[r_lo[c]:r_hi[c]]
        n0p[c] = pack_cols(nn.astype(BF16), 8)

        gl = np.full(NP, -1.0, np.float32)
        gl[:nc_n] = node_graph[r_lo[c]:r_hi[c]].astype(np.float32)
        rlocN8[c] = (gl.reshape(NP // 1024, 128, 8).transpose(1, 0, 2)
                     .reshape(128, -1).astype(BF16))
        rlocN4[c] = (gl.reshape(NP // 512, 128, 4).transpose(1, 0, 2)
                     .reshape(128, -1).astype(BF16))

    # ---- weights / biases --------------------------------------------------
    L = SELU_L
    W2p, b2p = pn[1]; W3p, b3p = pn[2]; W4p, b4p = pn[3]; W5p, b5p = pn[4]
    W1e_u, W1n_u, b1u = ue[0][0][:3], ue[0][0][3:], ue[0][1]
    W2u, b2u = ue[1]; W3u, b3u = ue[2]; W4u, b4u = ue[3]; W5u, b5u = ue[4]
    Wc1 = W5p @ W1n_u
    bias_z1f = b5p @ W1n_u + b1u
    bias_z1 = bias_z1f + b5u @ W1e_u

    W1pe, b1pe = pe[0]; W2pe, b2pe = pe[1]; W3pe, b3pe = pe[2]
    W4pe, b4pe = pe[3]; W5pe, b5pe = pe[4]
    W1e_n, W1n_n, b1n = un[0][0][:3], un[0][0][3:], un[0][1]
    W2un, b2un = un[1]; W3un, b3un = un[2]; W4un, b4un = un[3]; W5un, b5un = un[4]
    Wc2 = W5pe @ W1n_n
    bias_nz1f = b5pe @ W1n_n + b1n
    bias_nz1 = bias_nz1f + b5un @ W1e_n

    Wp1, bp1 = pr[0]; Wp2, bp2 = pr[1]; Wp3, bp3 = pr[2]
    Wp4, bp4 = pr[3]; Wp5, bp5 = pr[4]

    def bd16(W, b_):
        return blockdiag(W, b_).astype(BF16)

    weights = {
        "w_p2": bd16(W2p, 4), "w_p3": bd16(L * W3p, 4),
        "w_p4": bd16(L * W4p, 4), "w_pc1": bd16(L * Wc1, 4),
        "w_e1": bd16(W1e_u, 8), "w_id12": bd16(np.eye(12, dtype=np.float32), 8),
        "w_e2": bd16(L * W2u, 8), "w_e3": bd16(L * W3u, 8),
        "w_e4": bd16(L * W4u, 8), "w_e5": bd16(L * W5u, 8),
        "w_n1": bd16(W1pe, 4), "w_n2": bd16(L * W2pe, 4),
        "w_n3": bd16(L * W3pe, 4), "w_n4": bd16(L * W4pe, 4),
        "w_nc2": bd16(L * Wc2, 4), "w_nh": bd16(L * W5pe, 4),
        "w_m1": bd16(W1e_n, 8), "w_mid": bd16(np.eye(12, dtype=np.float32), 8),
        "w_m2": bd16(L * W2un, 8), "w_m3": bd16(L * W3un, 8),
        "w_m4": bd16(L * W4un, 8), "w_m5": bd16(L * W5un, 8),
        "w_q1n": Wp1[:3].astype(BF16), "w_q1e": Wp1[3:].astype(BF16),
        "w_q2": (L * Wp2).astype(BF16), "w_q3": (L * Wp3).astype(BF16),
        "w_q4": (L * Wp4).astype(BF16), "w_q5": (L * Wp5).astype(BF16),
    }

    biases = {}

    def add_bias(name, vec, b_):
        v = rep_bias(vec, b_)
        biases[name + "_e"] = (v + LN_A).astype(np.float32)
        biases[name] = v.astype(np.float32)

    add_bias("b_p2", b2p, 4); add_bias("b_p3", b3p, 4); add_bias("b_p4", b4p, 4)
    add_bias("b_z1f", bias_z1f, 8); add_bias("b_z1", bias_z1, 8)
    add_bias("b_u2", b2u, 8); add_bias("b_u3", b3u, 8); add_bias("b_u4", b4u, 8)
    biases["b_u5w"] = rep_bias(b5u, 8)
    add_bias("b_n1", b1pe, 4); add_bias("b_n2", b2pe, 4)
    add_bias("b_n3", b3pe, 4); add_bias("b_n4", b4pe, 4)
    biases["b_nh"] = rep_bias(b5pe, 4)
    add_bias("b_nz1f", bias_nz1f, 8); add_bias("b_nz1", bias_nz1, 8)
    add_bias("b_m2", b2un, 8); add_bias("b_m3", b3un, 8); add_bias("b_m4", b4un, 8)
    biases["b_m5w"] = rep_bias(b5un, 8)
    add_bias("b_q1", bp1, 1); add_bias("b_q2", bp2, 1)
    add_bias("b_q3", bp3, 1); add_bias("b_q4", bp4, 1)
    biases["b_q5w"] = rep_bias(bp5, 1)

    consts = {
        "iotaE": np.tile(np.arange(WIN, dtype=np.float32), (128, 8)).astype(BF16),
        "iotaG": np.tile(np.arange(G, dtype=np.float32), (128, 8)).astype(BF16),
        "iotaG4": np.tile(np.arange(G, dtype=np.float32), (128, 4)).astype(BF16),
        "eye24": np.eye(24, dtype=np.float32).astype(BF16),
        "eye64": np.eye(64, dtype=np.float32).astype(BF16),
    }

    cfg = dict(EP=EP, NP=NP, TCH=TCH, SUP=SUP, NSUP=NSUP,
               STRIDE=STRIDE, WIN=WIN, G=G,
               wshapes={k: v.shape for k, v in weights.items()},
               bshapes={k: v.shape for k, v in biases.items()},
               cshapes={k: v.shape for k, v in consts.items()})

    nc = _build(cfg)

    common = {}
    common.update(weights); common.update(biases); common.update(consts)
    in_maps = []
    for c in range(NCORE):
        m = dict(common)
        m.update(s1p=np.asarray(s1p[c]), e0p=np.asarray(e0p[c]),
                 rlocE=np.asarray(rlocE[c]), n0p=np.asarray(n0p[c]),
                 rlocN8=np.asarray(rlocN8[c]), rlocN4=np.asarray(rlocN4[c]))
        in_maps.append(m)

    import os
    from concourse.bass_utils import run_bass_kernel_spmd
    trace = bool(int(os.environ.get("KERNEL_TRACE", "0")))
    res = run_bass_kernel_spmd(nc, in_maps, core_ids=list(range(NCORE)),
                               trace=trace)
    last_run_info["exec_time_ns"] = res.exec_time_ns

    out = np.asarray(res.results[0]["out"], np.float32)
    return np.ascontiguousarray(out[:9].T)


# --------------------------------------------------------------- device IR

def _build(cfg):
    import concourse.bass as bass
    import concourse.tile as tile
    from concourse import bacc, mybir
    from contextlib import ExitStack

    F32, BF = mybir.dt.float32, mybir.dt.bfloat16
    AF = mybir.ActivationFunctionType
    OP = mybir.AluOpType

    EP, NP = cfg["EP"], cfg["NP"]
    TCH, SUP, NSUP = cfg["TCH"], cfg["SUP"], cfg["NSUP"]
    STRIDE, WIN, G = cfg["STRIDE"], cfg["WIN"], cfg["G"]

    SE = SUP * 1024
    C8, C4 = SE // 8, SE // 4
    NB8, NB4 = NP // 8, NP // 4
    CH = 1024
    OV = WIN - STRIDE          # window overlap (72)
    NPW = STRIDE * TCH + WIN

    nc = bacc.Bacc("TRN2", target_bir_lowering=False, debug=False,
                   num_devices=8)

    dram = {}
    for k, shp in cfg["wshapes"].items():
        dram[k] = nc.declare_dram_parameter(k, list(shp), BF, isOutput=False)
    for k, shp in cfg["bshapes"].items():
        dram[k] = nc.declare_dram_parameter(k, list(shp), F32, isOutput=False)
    for k, shp in cfg["cshapes"].items():
        dram[k] = nc.declare_dram_parameter(k, list(shp), BF, isOutput=False)
    for k, shp in [("s1p", [64, EP // 4]), ("e0p", [24, EP // 8]),
                   ("rlocE", [128, 8 * TCH]), ("n0p", [24, NP // 8]),
                   ("rlocN8", [128, 8 * (NP // 1024)]),
                   ("rlocN4", [128, 4 * (NP // 512)])]:
        dram[k] = nc.declare_dram_parameter(k, shp, BF, isOutput=False)
    out_ext = nc.declare_dram_parameter("out", [12, G], F32, isOutput=True)

    with tile.TileContext(nc) as tc, ExitStack() as ctx:
        const = ctx.enter_context(tc.tile_pool(name="const", bufs=1))
        sup_p = ctx.enter_context(tc.tile_pool(name="sup", bufs=2))
        work = ctx.enter_context(tc.tile_pool(name="work", bufs=3))
        pers = ctx.enter_context(tc.tile_pool(name="pers", bufs=1))
        work1 = ctx.enter_context(tc.tile_pool(name="work1", bufs=1))
        ps = ctx.enter_context(tc.tile_pool(name="ps", bufs=1, space="PSUM"))
        pssm = ctx.enter_context(tc.tile_pool(name="pssm", bufs=2, space="PSUM"))
        dr = ctx.enter_context(tc.tile_pool(name="dr", bufs=1, space="DRAM"))

        W = {}
        for k in (list(cfg["wshapes"]) + list(cfg["bshapes"])
                  + list(cfg["cshapes"])):
            shp = (cfg["wshapes"].get(k) or cfg["bshapes"].get(k)
                   or cfg["cshapes"][k])
            dt = F32 if k in cfg["bshapes"] else BF
            t = const.tile(list(shp), dt, tag=k)
            nc.sync.dma_start(t[:], dram[k][:])
            W[k] = t
        rlocN8s = const.tile([128, 8 * (NP // 1024)], BF, tag="rlocN8")
        nc.sync.dma_start(rlocN8s[:], dram["rlocN8"][:])
        rlocN4s = const.tile([128, 4 * (NP // 512)], BF, tag="rlocN4")
        nc.sync.dma_start(rlocN4s[:], dram["rlocN4"][:])

        partials_d = dr.tile([3, NPW], F32, tag="partials")

        # ----------------- helpers -----------------------------------------
        def selu_stage(z, cols, bias, relu_on_act, tag_p=""):
            rows = z.shape[0]
            Et = work.tile([rows, cols], BF, tag="sE")
            nc.scalar.activation(Et[:], z[:], AF.Exp,
                                 bias=W[bias + "_e"][0:rows, :], scale=1.0)
            t = work.tile([rows, cols], BF, tag="st")
            if relu_on_act:
                nc.scalar.activation(t[:], z[:], AF.Relu,
                                     bias=W[bias][0:rows, :], scale=1.0)
            else:
                nc.vector.tensor_scalar(t[:], z[:], W[bias][0:rows, :], 0.0,
                                        OP.add, OP.max)
            q = work.tile([rows, cols], BF, tag="sq")
            nc.vector.tensor_scalar(q[:], Et[:], float(SELU_A), 0.0,
                                    OP.subtract, OP.min)
            return t, q

        def mm_tq(zt, wname, t, q, cols):
            st = W[wname]
            for o in range(0, cols, 512):
                sl = slice(o, o + 512)
                nc.tensor.matmul(zt[:, sl], st[:], t[:, sl], start=True, stop=False)
                nc.tensor.matmul(zt[:, sl], st[:], q[:, sl], start=False, stop=True)

        def mm_one(zt, wname, x, cols):
            for o in range(0, cols, 512):
                sl = slice(o, o + 512)
                nc.tensor.matmul(zt[:, sl], W[wname][:], x[:, sl],
                                 start=True, stop=True)

        def mm_pair(zt, wa, xa, wb, xb, cols):
            for o in range(0, cols, 512):
                sl = slice(o, o + 512)
                nc.tensor.matmul(zt[:, sl], W[wa][:], xa[:, sl],
                                 start=True, stop=False)
                nc.tensor.matmul(zt[:, sl], W[wb][:], xb[:, sl],
                                 start=False, stop=True)

        def upd_loop(e_buf, c1, hout, bias1, biash, wpfx, cols, bwname):
            for it in range(10):
                b_l1 = bias1 + ("f" if it == 0 else "")
                for o8 in range(0, cols, CH):
                    sl8 = slice(o8, o8 + CH)
                    z1 = ps.tile([96, CH], F32, tag="zA")
                    mm_pair(z1, wpfx + "1", e_buf[:, sl8],
                            "w_id12" if wpfx == "w_e" else "w_mid",
                            c1[:, sl8], CH)
                    t1, q1 = selu_stage(z1, CH, b_l1, True)
                    z2 = ps.tile([96, CH], F32, tag="zB")
                    mm_tq(z2, wpfx + "2", t1, q1, CH)
                    t2, q2 = selu_stage(z2, CH, biash + "2", False)
                    z3 = ps.tile([96, CH], F32, tag="zA")
                    mm_tq(z3, wpfx + "3", t2, q2, CH)
                    t3, q3 = selu_stage(z3, CH, biash + "3", True)
                    z4 = ps.tile([64, CH], F32, tag="zB")
                    mm_tq(z4, wpfx + "4", t3, q3, CH)
                    t4, q4 = selu_stage(z4, CH, biash + "4", False)
                    z5 = ps.tile([24, CH], F32, tag="zA")
                    mm_tq(z5, wpfx + "5", t4, q4, CH)
                    if it < 9:
                        nc.vector.tensor_scalar(e_buf[:, sl8], z5[:],
                                                0.0, None, OP.add)
                    else:
                        nc.vector.tensor_scalar(hout[:, sl8], z5[:],
                                                W[bwname][0:24, :], None,
                                                OP.add)

        # ----------------- edge phase ---------------------------------------
        iotaE3 = W["iotaE"][:].rearrange("p (a b) -> p a b", b=WIN)
        Rprev = work.tile([3, WIN], F32, tag="R0")
        nc.vector.memset(Rprev[:], 0.0)

        for sup in range(NSUP):
            s1t = sup_p.tile([64, C4], BF, tag="s1t")
            nc.sync.dma_start(s1t[:], dram["s1p"][:, sup * C4:(sup + 1) * C4])
            e_buf = sup_p.tile([24, C8], BF, tag="e_buf")
            nc.sync.dma_start(e_buf[:], dram["e0p"][:, sup * C8:(sup + 1) * C8])
            rle = sup_p.tile([128, 8 * SUP], BF, tag="rle")
            nc.sync.dma_start(rle[:],
                              dram["rlocE"][:, sup * 8 * SUP:(sup + 1) * 8 * SUP])

            c1_4 = sup_p.tile([48, C4], BF, tag="c1_4")
            for o4 in range(0, C4, CH):
                sl4 = slice(o4, o4 + CH)
                z2 = ps.tile([128, CH], F32, tag="zA")
                mm_one(z2, "w_p2", s1t[:, sl4], CH)
                t2, q2 = selu_stage(z2, CH, "b_p2", True)
                z3 = ps.tile([128, CH], F32, tag="zB")
                mm_tq(z3, "w_p3", t2, q2, CH)
                t3, q3 = selu_stage(z3, CH, "b_p3", False)
                z4 = ps.tile([128, CH], F32, tag="zA")
                mm_tq(z4, "w_p4", t3, q3, CH)
                t4, q4 = selu_stage(z4, CH, "b_p4", True)
                zc = ps.tile([48, CH], F32, tag="zB")
                mm_tq(zc, "w_pc1", t4, q4, CH)
                nc.vector.tensor_scalar(c1_4[:, sl4], zc[:], 0.0, None, OP.add)

            c1 = sup_p.tile([96, C8], BF, tag="c1")
            for j8 in range(8):
                nc.sync.dma_start(
                    c1[12 * j8:12 * j8 + 12, :],
                    c1_4[12 * (j8 % 4):12 * (j8 % 4) + 12, (j8 // 4)::2])

            h1 = sup_p.tile([24, C8], BF, tag="h1")
            upd_loop(e_buf, c1, h1, "b_z1", "b_u", "w_e", C8, "b_u5w")

            fbuf = sup_p.tile([3, SUP * STRIDE], F32, tag="fbuf")
            for ch in range(SUP):
                cs = slice(ch * 128, (ch + 1) * 128)
                tp = pssm.tile([128, 64], BF, tag="tp")
                nc.tensor.transpose(tp[:, 0:24], h1[:, cs], W["eye24"][:])
                T1 = work.tile([128, 24], BF, tag="T1")
                nc.vector.tensor_scalar(T1[:], tp[:, 0:24], 0.0, None, OP.add)
                oh = work.tile([128, 8 * WIN], BF, tag="oh")
                nc.vector.scalar_tensor_tensor(
                    oh[:].rearrange("p (a b) -> p a b", b=WIN),
                    iotaE3, 1.0,
                    rle[:, ch * 8:(ch + 1) * 8].broadcast_to([128, 8, WIN]),
                    OP.mult, OP.is_equal)
                gps = pssm.tile([3, WIN], F32, tag="gps")
                for j in range(8):
                    nc.tensor.matmul(gps[:], T1[:, 3 * j:3 * j + 3],
                                     oh[:, j * WIN:(j + 1) * WIN],
                                     start=(j == 0), stop=(j == 7))
                Rnew = work.tile([3, WIN], F32, tag="R0")
                nc.vector.tensor_tensor(Rnew[:, 0:OV], gps[:, 0:OV],
                                        Rprev[:, STRIDE:WIN], OP.add)
                nc.vector.tensor_scalar(Rnew[:, OV:WIN], gps[:, OV:WIN],
                                        0.0, None, OP.add)
                nc.vector.tensor_scalar(fbuf[:, ch * STRIDE:(ch + 1) * STRIDE],
                                        Rprev[:, 0:STRIDE], 0.0, None, OP.add)
                Rprev = Rnew
            nc.sync.dma_start(
                partials_d[:, sup * SUP * STRIDE:(sup + 1) * SUP * STRIDE],
                fbuf[:])

        nc.sync.dma_start(partials_d[:, TCH * STRIDE:TCH * STRIDE + WIN],
                          Rprev[:])

        # ----------------- node phase ---------------------------------------
        c2_4 = pers.tile([48, NB4], BF, tag="c2_4")
        h2e4 = pers.tile([64, NB4], BF, tag="h2e4")

        for o4 in range(0, NB4, CH):
            sl4 = slice(o4, o4 + CH)
            pch = work1.tile([3, 4 * CH], F32, tag="pch")
            nc.sync.dma_start(pch[:], partials_d[:, 4 * o4:4 * o4 + 4 * CH])
            pbf = work1.tile([3, 4 * CH], BF, tag="pbf")
            nc.vector.tensor_scalar(pbf[:], pch[:], 0.0, None, OP.add)
            pn4 = work.tile([12, CH], BF, tag="pn4")
            for j in range(4):
                nc.sync.dma_start(pn4[3 * j:3 * j + 3, :], pbf[:, j::4])

            z1 = ps.tile([64, CH], F32, tag="zA")
            mm_one(z1, "w_n1", pn4, CH)
            t1, q1 = selu_stage(z1, CH, "b_n1", True)
            z2 = ps.tile([128, CH], F32, tag="zB")
            mm_tq(z2, "w_n2", t1, q1, CH)
            t2, q2 = selu_stage(z2, CH, "b_n2", False)
            z3 = ps.tile([128, CH], F32, tag="zA")
            mm_tq(z3, "w_n3", t2, q2, CH)
            t3, q3 = selu_stage(z3, CH, "b_n3", True)
            z4 = ps.tile([128, CH], F32, tag="zB")
            mm_tq(z4, "w_n4", t3, q3, CH)
            t4, q4 = selu_stage(z4, CH, "b_n4", False)
            zc = ps.tile([48, CH], F32, tag="zA")
            mm_tq(zc, "w_nc2", t4, q4, CH)
            nc.vector.tensor_scalar(c2_4[:, sl4], zc[:], 0.0, None, OP.add)
            zh = ps.tile([64, CH], F32, tag="zB")
            mm_tq(zh, "w_nh", t4, q4, CH)
            nc.vector.tensor_scalar(h2e4[:, sl4], zh[:],
                                    W["b_nh"][0:64, :], None, OP.add)

        c2 = pers.tile([96, NB8], BF, tag="c2")
        for j8 in range(8):
            nc.sync.dma_start(
                c2[12 * j8:12 * j8 + 12, :],
                c2_4[12 * (j8 % 4):12 * (j8 % 4) + 12, (j8 // 4)::2])

        nbuf = pers.tile([24, NB8], BF, tag="nbuf")
        nc.sync.dma_start(nbuf[:], dram["n0p"][:])
        h2n = pers.tile([24, NB8], BF, tag="h2n")
        upd_loop(nbuf, c2, h2n, "b_nz1", "b_m", "w_m", NB8, "b_m5w")

        # graph segment sums
        gn_acc = pers.tile([3, G], F32, tag="gn_acc")
        nc.vector.memset(gn_acc[:], 0.0)
        ge_acc = pers.tile([16, G], F32, tag="ge_acc")
        nc.vector.memset(ge_acc[:], 0.0)
        iotaG3 = W["iotaG"][:].rearrange("p (a b) -> p a b", b=G)
        iotaG43 = W["iotaG4"][:].rearrange("p (a b) -> p a b", b=G)

        for ch in range(NP // 1024):
            cs = slice(ch * 128, (ch + 1) * 128)
            tp = pssm.tile([128, 64], BF, tag="tp")
            nc.tensor.transpose(tp[:, 0:24], h2n[:, cs], W["eye24"][:])
            T1 = work.tile([128, 24], BF, tag="T1")
            nc.vector.tensor_scalar(T1[:], tp[:, 0:24], 0.0, None, OP.add)
            oh = work.tile([128, 8 * G], BF, tag="oh")
            nc.vector.scalar_tensor_tensor(
                oh[:].rearrange("p (a b) -> p a b", b=G), iotaG3, 1.0,
                rlocN8s[:, ch * 8:(ch + 1) * 8].broadcast_to([128, 8, G]),
                OP.mult, OP.is_equal)
            gq = pssm.tile([3, G], F32, tag="gps")
            for j in range(8):
                nc.tensor.matmul(gq[:], T1[:, 3 * j:3 * j + 3],
                                 oh[:, j * G:(j + 1) * G],
                                 start=(j == 0), stop=(j == 7))
            nc.vector.tensor_tensor(gn_acc[:], gq[:], gn_acc[:], OP.add)

        for ch in range(NP // 512):
            cs = slice(ch * 128, (ch + 1) * 128)
            tp = pssm.tile([128, 64], BF, tag="tp")
            nc.tensor.transpose(tp[:], h2e4[:, cs], W["eye64"][:])
            T1 = work.tile([128, 64], BF, tag="T14")
            nc.vector.tensor_scalar(T1[:], tp[:], 0.0, None, OP.add)
            oh = work.tile([128, 4 * G], BF, tag="oh")
            nc.vector.scalar_tensor_tensor(
                oh[:, 0:4 * G].rearrange("p (a b) -> p a b", b=G), iotaG43, 1.0,
                rlocN4s[:, ch * 4:(ch + 1) * 4].broadcast_to([128, 4, G]),
                OP.mult, OP.is_equal)
            gq = pssm.tile([16, G], F32, tag="gps")
            for j in range(4):
                nc.tensor.matmul(gq[:], T1[:, 16 * j:16 * j + 16],
                                 oh[:, j * G:(j + 1) * G],
                                 start=(j == 0), stop=(j == 3))
            nc.vector.tensor_tensor(ge_acc[:], gq[:], ge_acc[:], OP.add)

        # AllReduce + predict
        bounce_in = dr.tile([19, G], F32, tag="bin")
        bounce_out = dr.tile([19, G], F32, tag="bout")
        nc.gpsimd.dma_start(bounce_in[0:3, :], gn_acc[:])
        nc.gpsimd.dma_start(bounce_in[3:19, :], ge_acc[:])
        nc.gpsimd.collective_compute(
            "AllReduce", mybir.AluOpType.add, replica_groups=[list(range(8))],
            ins=[bounce_in.opt()], outs=[bounce_out.opt()])
        gr_n = pers.tile([3, G], BF, tag="gr_n")
        gr_nf = pers.tile([3, G], F32, tag="gr_nf")
        nc.sync.dma_start(gr_nf[:], bounce_out[0:3, :])
        nc.vector.tensor_scalar(gr_n[:], gr_nf[:], 0.0, None, OP.add)
        gr_e = pers.tile([16, G], BF, tag="gr_e")
        gr_ef = pers.tile([16, G], F32, tag="gr_ef")
        nc.sync.dma_start(gr_ef[:], bounce_out[3:19, :])
        nc.vector.tensor_scalar(gr_e[:], gr_ef[:], 0.0, None, OP.add)

        z1 = pssm.tile([12, G], F32, tag="gps")
        nc.tensor.matmul(z1[:], W["w_q1n"][:], gr_n[:], start=True, stop=False)
        nc.tensor.matmul(z1[:], W["w_q1e"][:], gr_e[:], start=False, stop=True)
        t, q = selu_stage(z1, G, "b_q1", True)
        for li in (2, 3, 4):
            zi = pssm.tile([12, G], F32, tag="gps")
            nc.tensor.matmul(zi[0:(12 if li < 4 else 8), :],
                             W[f"w_q{li}"][:], t[:], start=True, stop=False)
            nc.tensor.matmul(zi[0:(12 if li < 4 else 8), :],
                             W[f"w_q{li}"][:], q[:], start=False, stop=True)
            t, q = selu_stage(zi[0:(12 if li < 4 else 8), :], G,
                              f"b_q{li}", True)
        z5 = pssm.tile([9, G], F32, tag="gps")
        nc.tensor.matmul(z5[:], W["w_q5"][:], t[:], start=True, stop=False)
        nc.tensor.matmul(z5[:], W["w_q5"][:], q[:], start=False, stop=True)
        outt = pers.tile([12, G], F32, tag="outt")
        nc.vector.memset(outt[:], 0.0)
        nc.vector.tensor_scalar(outt[0:9, :], z5[:], W["b_q5w"][0:9, :],
                                None, OP.add)
        nc.sync.dma_start(out_ext[:], outt[:])

    nc.compile()
    return nc


# revision 9
# speedup vs baseline: 1.8176x; 1.8176x over previous
"""Trainium2 Bass kernel for GNN message passing (8 NeuronCores, SPMD).

Sharding: edges sorted by receiver, sharded across 8 cores at receiver
boundaries -> each core owns a disjoint receiver range; only the tiny
[19, G] graph partials are AllReduced.

Uniform b=8 block-diagonal column packing everywhere (32-wide layers are
split into 16-wide halves with accumulating matmuls), so no on-device
layout regroups are needed. Node arrays use a block distribution
(node = slot*NB8 + col) so partials stream back from DRAM contiguously.

SELU exact: with E = exp(y + ln a), selu(y) = l*(relu(y) + min(E-a, 0));
t/q branches kept in fp32 and fed as K-split accumulate matmuls.
"""

import math
import numpy as np
import ml_dtypes

SELU_L = 1.0507009873554805
SELU_A = 1.6732632423543772
LN_A = math.log(SELU_A)
BF16 = ml_dtypes.bfloat16

last_run_info = {}


def _np(x, dt=None):
    a = np.asarray(x)
    return a.astype(dt) if dt is not None else a


def selu_np(x):
    return SELU_L * np.where(x > 0, x, SELU_A * (np.exp(np.minimum(x, 0)) - 1.0))


def pack_cols(arr, b):
    """[E, w] -> [w*b, E/b]: out[w*j + f, c] = arr[b*c + j, f]."""
    E, w = arr.shape
    return np.ascontiguousarray(
        arr.reshape(E // b, b, w).transpose(1, 2, 0).reshape(b * w, E // b))


def pack_block(arr, nb):
    """[NP, w] -> [w*8, NP/8] block distribution:
    out[w*j + f, c] = arr[j*nb + c, f]."""
    NP, w = arr.shape
    return np.ascontiguousarray(
        arr.reshape(8, nb, w).transpose(0, 2, 1).reshape(8 * w, nb))


def blockdiag(W, b):
    i, o = W.shape
    out = np.zeros((i * b, o * b), dtype=np.float32)
    for k in range(b):
        out[k * i:(k + 1) * i, k * o:(k + 1) * o] = W
    return out


def rep_bias(bvec, b):
    return np.tile(np.asarray(bvec, np.float32), b).reshape(-1, 1)


def _pack_chunks(recv_rel, stride, cap, win):
    n = len(recv_rel)
    chunk_of = np.empty(n, np.int32)
    pos, ch = 0, 0
    while pos < n:
        base = stride * ch
        if recv_rel[pos] < base:
            return None
        hi_mand = np.searchsorted(recv_rel, base + stride, side="left")
        hi_opt = np.searchsorted(recv_rel, base + win, side="left")
        if hi_mand - pos > cap:
            return None
        take = max(min(hi_opt - pos, cap), hi_mand - pos)
        chunk_of[pos:pos + take] = ch
        pos += take
        ch += 1
    return chunk_of, ch


def kernel(**inputs):
    nodes = _np(inputs["nodes"], np.float32)
    edges = _np(inputs["edges"], np.float32)
    receivers = _np(inputs["receivers"], np.int64)
    senders = _np(inputs["senders"], np.int64)
    node_graph = _np(inputs["node_graph"], np.int64)
    G = int(np.asarray(inputs["num_graphs"]))
    params = inputs["params"]

    def lay(name):
        return [(np.asarray(W, np.float32), np.asarray(b, np.float32))
                for (W, b) in params[name]]

    pn, ue, pe, un, pr = (lay(k) for k in
                          ("perm_nodes", "upd_edges", "perm_edges",
                           "upd_nodes", "predict"))

    N, E = nodes.shape[0], edges.shape[0]
    NCORE = 8
    CAP, WIN, STRIDE, SUP = 1024, 128, 56, 16

    # ---- host: perm_nodes layer 1 (factorized gather + selu) --------------
    W1, b1 = pn[0]
    A = nodes @ W1[:3] + b1
    Bn = nodes @ W1[3:]
    s1_full = selu_np(A[receivers] + Bn[senders]).astype(BF16)

    # ---- shard by receiver range ------------------------------------------
    order = np.argsort(receivers, kind="stable")
    recv_s = receivers[order]
    cuts = [0]
    for i in range(1, NCORE):
        tgt = i * E // NCORE
        cuts.append(int(np.searchsorted(recv_s, recv_s[min(tgt, E - 1)],
                                        side="left")))
    cuts.append(E)
    r_lo = [int(recv_s[cuts[i]]) if cuts[i] < E else N for i in range(NCORE)]
    r_hi = r_lo[1:] + [N]

    per_core = []
    for c in range(NCORE):
        lo, hi = cuts[c], cuts[c + 1]
        rrel = (recv_s[lo:hi] - r_lo[c]).astype(np.int64)
        res = None
        for st in (STRIDE, 48, 40, 32):
            res = _pack_chunks(rrel, st, CAP, WIN)
            if res is not None:
                STRIDE = min(STRIDE, st)
                break
        assert res is not None
        per_core.append([order[lo:hi], rrel, res[0], res[1]])
    for c in range(NCORE):
        res = _pack_chunks(per_core[c][1], STRIDE, CAP, WIN)
        assert res is not None
        per_core[c][2], per_core[c][3] = res

    NC_nodes = max(h - l for l, h in zip(r_lo, r_hi))
    NP = ((max(NC_nodes, 1) + 8191) // 8192) * 8192
    NB8 = NP // 8
    TCH = max(pc[3] for pc in per_core)
    TCH = ((TCH + SUP - 1) // SUP) * SUP
    EP = TCH * CAP
    NSUP = TCH // SUP

    # ---- per-core packed arrays -------------------------------------------
    s1p = np.zeros((NCORE, 128, EP // 8), BF16)
    e0p = np.zeros((NCORE, 24, EP // 8), np.float32)
    rlocE = np.full((NCORE, 128, 8 * TCH), -1.0, BF16)
    n0p = np.zeros((NCORE, 24, NB8), np.float32)
    rlocN8 = np.full((NCORE, 128, 8 * (NP // 1024)), -1.0, BF16)

    for c in range(NCORE):
        idx, rrel, chunk_of, tch = per_core[c]
        ne = len(idx)
        if ne:
            ch_starts = np.searchsorted(chunk_of, np.arange(chunk_of.max() + 1))
            pos = np.arange(ne) - ch_starts[chunk_of]
            slot = chunk_of.astype(np.int64) * CAP + pos
        else:
            slot = np.zeros(0, np.int64)

        se = np.zeros((EP, 16), BF16)
        se[slot] = s1_full[idx]
        s1p[c] = pack_cols(se, 8)

        ee = np.zeros((EP, 3), np.float32)
        ee[slot] = edges[idx]
        e0p[c] = pack_cols(ee, 8)

        rl = np.full(EP, -1.0, np.float32)
        rl[slot] = (rrel - np.int64(STRIDE) * chunk_of).astype(np.float32)
        rlocE[c] = (rl.reshape(TCH, 128, 8).transpose(1, 0, 2)
                    .reshape(128, TCH * 8).astype(BF16))

        nc_n = r_hi[c] - r_lo[c]
        nn = np.zeros((NP, 3), np.float32)
        nn[:nc_n] = nodes[r_lo[c]:r_hi[c]]
        n0p[c] = pack_block(nn, NB8)

        gl = np.full(NP, -1.0, np.float32)
        gl[:nc_n] = node_graph[r_lo[c]:r_hi[c]].astype(np.float32)
        gl2 = gl.reshape(8, NB8)
        # rlocN8[cc, 8*ch + j] = gl2[j, ch*128 + cc]
        rlocN8[c] = (gl2.reshape(8, NB8 // 128, 128).transpose(2, 1, 0)
                     .reshape(128, -1).astype(BF16))

    # ---- weights / biases --------------------------------------------------
    L = SELU_L
    W2p, b2p = pn[1]; W3p, b3p = pn[2]; W4p, b4p = pn[3]; W5p, b5p = pn[4]
    W1e_u, W1n_u, b1u = ue[0][0][:3], ue[0][0][3:], ue[0][1]
    W2u, b2u = ue[1]; W3u, b3u = ue[2]; W4u, b4u = ue[3]; W5u, b5u = ue[4]
    Wc1 = W5p @ W1n_u
    bias_z1f = b5p @ W1n_u + b1u
    bias_z1 = bias_z1f + b5u @ W1e_u

    W1pe, b1pe = pe[0]; W2pe, b2pe = pe[1]; W3pe, b3pe = pe[2]
    W4pe, b4pe = pe[3]; W5pe, b5pe = pe[4]
    W1e_n, W1n_n, b1n = un[0][0][:3], un[0][0][3:], un[0][1]
    W2un, b2un = un[1]; W3un, b3un = un[2]; W4un, b4un = un[3]; W5un, b5un = un[4]
    Wc2 = W5pe @ W1n_n
    bias_nz1f = b5pe @ W1n_n + b1n
    bias_nz1 = bias_nz1f + b5un @ W1e_n

    Wp1, bp1 = pr[0]; Wp2, bp2 = pr[1]; Wp3, bp3 = pr[2]
    Wp4, bp4 = pr[3]; Wp5, bp5 = pr[4]

    def bd(Wm):
        return blockdiag(Wm, 8).astype(np.float32)

    def bdb(Wm):
        return blockdiag(Wm, 8).astype(BF16)

    weights = {}   # fp32 stationaries
    wbf = {}       # bf16 stationaries

    wbf["w_p2_l"] = bdb(W2p[:, :16]); wbf["w_p2_h"] = bdb(W2p[:, 16:])
    for nm, Wm in (("p3", L * W3p), ("p4", L * W4p)):
        weights[f"w_{nm}_ll"] = bd(Wm[:16, :16])
        weights[f"w_{nm}_lh"] = bd(Wm[:16, 16:])
        weights[f"w_{nm}_hl"] = bd(Wm[16:, :16])
        weights[f"w_{nm}_hh"] = bd(Wm[16:, 16:])
    weights["w_pc1_l"] = bd(L * Wc1[:16]); weights["w_pc1_h"] = bd(L * Wc1[16:])

    weights["w_e1"] = bd(W1e_u)
    wbf["w_id12"] = bdb(np.eye(12, dtype=np.float32))
    weights["w_e2"] = bd(L * W2u); weights["w_e3"] = bd(L * W3u)
    weights["w_e4"] = bd(L * W4u); weights["w_e5"] = bd(L * W5u)

    weights["w_n1"] = bd(W1pe)
    weights["w_n2_l"] = bd(L * W2pe[:, :16])
    weights["w_n2_h"] = bd(L * W2pe[:, 16:])
    for nm, Wm in (("n3", L * W3pe), ("n4", L * W4pe)):
        weights[f"w_{nm}_ll"] = bd(Wm[:16, :16])
        weights[f"w_{nm}_lh"] = bd(Wm[:16, 16:])
        weights[f"w_{nm}_hl"] = bd(Wm[16:, :16])
        weights[f"w_{nm}_hh"] = bd(Wm[16:, 16:])
    weights["w_nc2_l"] = bd(L * Wc2[:16]); weights["w_nc2_h"] = bd(L * Wc2[16:])
    weights["w_nh_l"] = bd(L * W5pe[:16]); weights["w_nh_h"] = bd(L * W5pe[16:])

    weights["w_m1"] = bd(W1e_n)
    weights["w_m2"] = bd(L * W2un); weights["w_m3"] = bd(L * W3un)
    weights["w_m4"] = bd(L * W4un); weights["w_m5"] = bd(L * W5un)

    weights["w_q1n"] = Wp1[:3].copy(); weights["w_q1e"] = Wp1[3:].copy()
    weights["w_q2"] = L * Wp2; weights["w_q3"] = L * Wp3
    weights["w_q4"] = L * Wp4; weights["w_q5"] = L * Wp5

    biases = {}

    def add_bias(name, vec, b_=8):
        v = rep_bias(vec, b_)
        biases[name + "_e"] = (v + LN_A).astype(np.float32)
        biases[name] = v.astype(np.float32)

    add_bias("b_p2l", b2p[:16]); add_bias("b_p2h", b2p[16:])
    add_bias("b_p3l", b3p[:16]); add_bias("b_p3h", b3p[16:])
    add_bias("b_p4l", b4p[:16]); add_bias("b_p4h", b4p[16:])
    add_bias("b_z1f", bias_z1f); add_bias("b_z1", bias_z1)
    add_bias("b_u2", b2u); add_bias("b_u3", b3u); add_bias("b_u4", b4u)
    biases["b_u5w"] = rep_bias(b5u, 8)
    add_bias("b_n1", b1pe)
    add_bias("b_n2l", b2pe[:16]); add_bias("b_n2h", b2pe[16:])
    add_bias("b_n3l", b3pe[:16]); add_bias("b_n3h", b3pe[16:])
    add_bias("b_n4l", b4pe[:16]); add_bias("b_n4h", b4pe[16:])
    biases["b_nhv"] = rep_bias(b5pe, 8)
    add_bias("b_nz1f", bias_nz1f); add_bias("b_nz1", bias_nz1)
    add_bias("b_m2", b2un); add_bias("b_m3", b3un); add_bias("b_m4", b4un)
    biases["b_m5w"] = rep_bias(b5un, 8)
    add_bias("b_q1", bp1, 1); add_bias("b_q2", bp2, 1)
    add_bias("b_q3", bp3, 1); add_bias("b_q4", bp4, 1)
    biases["b_q5w"] = rep_bias(bp5, 1)

    consts = {
        "iotaE": np.tile(np.arange(WIN, dtype=np.float32), (128, 8)).astype(BF16),
        "iotaG": np.tile(np.arange(G, dtype=np.float32), (128, 8)).astype(BF16),
        "eye24": np.eye(24, dtype=np.float32).astype(BF16),
        "eye128": np.eye(128, dtype=np.float32).astype(BF16),
    }

    cfg = dict(EP=EP, NP=NP, TCH=TCH, SUP=SUP, NSUP=NSUP,
               STRIDE=STRIDE, WIN=WIN, G=G,
               wshapes={k: v.shape for k, v in weights.items()},
               wbshapes={k: v.shape for k, v in wbf.items()},
               bshapes={k: v.shape for k, v in biases.items()},
               cshapes={k: v.shape for k, v in consts.items()})

    nc = _build(cfg)

    common = {}
    common.update(weights); common.update(wbf)
    common.update(biases); common.update(consts)
    in_maps = []
    for c in range(NCORE):
        m = dict(common)
        m.update(s1p=np.asarray(s1p[c]), e0p=np.asarray(e0p[c]),
                 rlocE=np.asarray(rlocE[c]), n0p=np.asarray(n0p[c]),
                 rlocN8=np.asarray(rlocN8[c]))
        in_maps.append(m)

    import os
    from concourse.bass_utils import run_bass_kernel_spmd
    trace = bool(int(os.environ.get("KERNEL_TRACE", "0")))
    res = run_bass_kernel_spmd(nc, in_maps, core_ids=list(range(NCORE)),
                               trace=trace)
    last_run_info["exec_time_ns"] = res.exec_time_ns

    out = np.asarray(res.results[0]["out"], np.float32)
    return np.ascontiguousarray(out[:9].T)


# --------------------------------------------------------------- device IR

def _build(cfg):
    import concourse.bass as bass
    import concourse.tile as tile
    from concourse import bacc, mybir
    from contextlib import ExitStack

    F32, BF = mybir.dt.float32, mybir.dt.bfloat16
    AF = mybir.ActivationFunctionType
    OP = mybir.AluOpType

    EP, NP = cfg["EP"], cfg["NP"]
    TCH, SUP, NSUP = cfg["TCH"], cfg["SUP"], cfg["NSUP"]
    STRIDE, WIN, G = cfg["STRIDE"], cfg["WIN"], cfg["G"]

    SE = SUP * 1024
    C8 = SE // 8
    NB8 = NP // 8
    CH = 1024
    OV = WIN - STRIDE
    NPW = max(STRIDE * TCH + WIN, NP + 1024)

    nc = bacc.Bacc("TRN2", target_bir_lowering=False, debug=False,
                   num_devices=8)

    dram = {}
    for k, shp in cfg["wshapes"].items():
        dram[k] = nc.declare_dram_parameter(k, list(shp), F32, isOutput=False)
    for k, shp in cfg["wbshapes"].items():
        dram[k] = nc.declare_dram_parameter(k, list(shp), BF, isOutput=False)
    for k, shp in cfg["bshapes"].items():
        dram[k] = nc.declare_dram_parameter(k, list(shp), F32, isOutput=False)
    for k, shp in cfg["cshapes"].items():
        dram[k] = nc.declare_dram_parameter(k, list(shp), BF, isOutput=False)
    dram["s1p"] = nc.declare_dram_parameter("s1p", [128, EP // 8], BF,
                                            isOutput=False)
    dram["e0p"] = nc.declare_dram_parameter("e0p", [24, EP // 8], F32,
                                            isOutput=False)
    dram["rlocE"] = nc.declare_dram_parameter("rlocE", [128, 8 * TCH], BF,
                                              isOutput=False)
    dram["n0p"] = nc.declare_dram_parameter("n0p", [24, NB8], F32,
                                            isOutput=False)
    dram["rlocN8"] = nc.declare_dram_parameter(
        "rlocN8", [128, 8 * (NP // 1024)], BF, isOutput=False)
    out_ext = nc.declare_dram_parameter("out", [12, G], F32, isOutput=True)

    with tile.TileContext(nc) as tc, ExitStack() as ctx:
        const = ctx.enter_context(tc.tile_pool(name="const", bufs=1))
        sup_p = ctx.enter_context(tc.tile_pool(name="sup", bufs=2))
        work = ctx.enter_context(tc.tile_pool(name="work", bufs=3))
        pers = ctx.enter_context(tc.tile_pool(name="pers", bufs=1))
        ps = ctx.enter_context(tc.tile_pool(name="ps", bufs=2, space="PSUM"))
        dr = ctx.enter_context(tc.tile_pool(name="dr", bufs=1, space="DRAM"))

        W = {}
        for k in (list(cfg["wshapes"]) + list(cfg["wbshapes"])
                  + list(cfg["bshapes"]) + list(cfg["cshapes"])):
            shp = (cfg["wshapes"].get(k) or cfg["wbshapes"].get(k)
                   or cfg["bshapes"].get(k) or cfg["cshapes"][k])
            dt = F32 if (k in cfg["wshapes"] or k in cfg["bshapes"]) else BF
            t = const.tile(list(shp), dt, tag=k)
            nc.sync.dma_start(t[:], dram[k][:])
            W[k] = t
        rlocN8s = const.tile([128, 8 * (NP // 1024)], BF, tag="rlocN8")
        nc.sync.dma_start(rlocN8s[:], dram["rlocN8"][:])

        partials_d = dr.tile([3, NPW], F32, tag="partials")

        def selu_stage(z, cols, bias, relu_on_act):
            rows = z.shape[0]
            Et = work.tile([rows, cols], F32, tag="sE")
            nc.scalar.activation(Et[:], z[:], AF.Exp,
                                 bias=W[bias + "_e"][0:rows, :], scale=1.0)
            t = work.tile([rows, cols], F32, tag="st")
            if relu_on_act:
                nc.scalar.activation(t[:], z[:], AF.Relu,
                                     bias=W[bias][0:rows, :], scale=1.0)
            else:
                nc.vector.tensor_scalar(t[:], z[:], W[bias][0:rows, :], 0.0,
                                        OP.add, OP.max)
            q = work.tile([rows, cols], F32, tag="sq")
            nc.vector.tensor_scalar(q[:], Et[:], float(SELU_A), 0.0,
                                    OP.subtract, OP.min)
            return t, q

        def mm_acc(zt, pairs, cols):
            for o in range(0, cols, 512):
                sl = slice(o, o + 512)
                n = len(pairs)
                for i, (wn, x) in enumerate(pairs):
                    nc.tensor.matmul(zt[:, sl], W[wn][:], x[:, sl],
                                     start=(i == 0), stop=(i == n - 1))

        def upd_loop(e_buf, c1, hout, bias1, biash, wpfx, idname, cols,
                     bwname):
            for it in range(10):
                b_l1 = bias1 + ("f" if it == 0 else "")
                for o8 in range(0, cols, CH):
                    sl8 = slice(o8, o8 + CH)
                    z1 = ps.tile([96, CH], F32, tag="zA")
                    mm_acc(z1, [(wpfx + "1", e_buf[:, sl8]),
                                (idname, c1[:, sl8])], CH)
                    t1, q1 = selu_stage(z1, CH, b_l1, True)
                    z2 = ps.tile([96, CH], F32, tag="zB")
                    mm_acc(z2, [(wpfx + "2", t1), (wpfx + "2", q1)], CH)
                    t2, q2 = selu_stage(z2, CH, biash + "2", False)
                    z3 = ps.tile([96, CH], F32, tag="zA")
                    mm_acc(z3, [(wpfx + "3", t2), (wpfx + "3", q2)], CH)
                    t3, q3 = selu_stage(z3, CH, biash + "3", True)
                    z4 = ps.tile([64, CH], F32, tag="zB")
                    mm_acc(z4, [(wpfx + "4", t3), (wpfx + "4", q3)], CH)
                    t4, q4 = selu_stage(z4, CH, biash + "4", False)
                    z5 = ps.tile([24, CH], F32, tag="zA")
                    mm_acc(z5, [(wpfx + "5", t4), (wpfx + "5", q4)], CH)
                    if it < 9:
                        nc.vector.tensor_scalar(e_buf[:, sl8], z5[:],
                                                0.0, None, OP.add)
                    else:
                        nc.vector.tensor_scalar(hout[:, sl8], z5[:],
                                                W[bwname][0:24, :], None,
                                                OP.add)

        def perm_chain(src, cols, pfx, outs):
            """16->(32->32->32)->outs chain at b=8 with 16-wide halves."""
            for o in range(0, cols, CH):
                sl = slice(o, o + CH)
                sv = src[:, sl]
                z2l = ps.tile([128, CH], F32, tag="zA")
                mm_acc(z2l, [(f"w_{pfx}2_l", sv)], CH)
                z2h = ps.tile([128, CH], F32, tag="zB")
                mm_acc(z2h, [(f"w_{pfx}2_h", sv)], CH)
                t2l, q2l = selu_stage(z2l, CH, f"b_{pfx}2l", True)
                t2h, q2h = selu_stage(z2h, CH, f"b_{pfx}2h", False)
                z3l = ps.tile([128, CH], F32, tag="zA")
                mm_acc(z3l, [(f"w_{pfx}3_ll", t2l), (f"w_{pfx}3_ll", q2l),
                             (f"w_{pfx}3_hl", t2h), (f"w_{pfx}3_hl", q2h)], CH)
                z3h = ps.tile([128, CH], F32, tag="zB")
                mm_acc(z3h, [(f"w_{pfx}3_lh", t2l), (f"w_{pfx}3_lh", q2l),
                             (f"w_{pfx}3_hh", t2h), (f"w_{pfx}3_hh", q2h)], CH)
                t3l, q3l = selu_stage(z3l, CH, f"b_{pfx}3l", True)
                t3h, q3h = selu_stage(z3h, CH, f"b_{pfx}3h", False)
                z4l = ps.tile([128, CH], F32, tag="zA")
                mm_acc(z4l, [(f"w_{pfx}4_ll", t3l), (f"w_{pfx}4_ll", q3l),
                             (f"w_{pfx}4_hl", t3h), (f"w_{pfx}4_hl", q3h)], CH)
                z4h = ps.tile([128, CH], F32, tag="zB")
                mm_acc(z4h, [(f"w_{pfx}4_lh", t3l), (f"w_{pfx}4_lh", q3l),
                             (f"w_{pfx}4_hh", t3h), (f"w_{pfx}4_hh", q3h)], CH)
                t4l, q4l = selu_stage(z4l, CH, f"b_{pfx}4l", True)
                t4h, q4h = selu_stage(z4h, CH, f"b_{pfx}4h", False)
                for zrows, wl, wh, writer in outs:
                    zz = ps.tile([zrows, CH], F32, tag="zA")
                    mm_acc(zz, [(wl, t4l), (wl, q4l), (wh, t4h), (wh, q4h)],
                           CH)
                    writer(zz, sl)

        # ----------------- edge phase ---------------------------------------
        iotaE3 = W["iotaE"][:].rearrange("p (a b) -> p a b", b=WIN)
        Rprev = work.tile([3, WIN], F32, tag="R0")
        nc.vector.memset(Rprev[:], 0.0)

        for sup in range(NSUP):
            s1t = sup_p.tile([128, C8], BF, tag="s1t")
            nc.sync.dma_start(s1t[:], dram["s1p"][:, sup * C8:(sup + 1) * C8])
            e_buf = sup_p.tile([24, C8], F32, tag="e_buf")
            nc.sync.dma_start(e_buf[:], dram["e0p"][:, sup * C8:(sup + 1) * C8])
            rle = sup_p.tile([128, 8 * SUP], BF, tag="rle")
            nc.sync.dma_start(
                rle[:], dram["rlocE"][:, sup * 8 * SUP:(sup + 1) * 8 * SUP])

            c1 = sup_p.tile([96, C8], BF, tag="c1")

            def wr_c1(zz, sl, c1=c1):
                nc.vector.tensor_scalar(c1[:, sl], zz[:], 0.0, None, OP.add)

            perm_chain(s1t, C8, "p", [(96, "w_pc1_l", "w_pc1_h", wr_c1)])

            h1 = sup_p.tile([24, C8], BF, tag="h1")
            upd_loop(e_buf, c1, h1, "b_z1", "b_u", "w_e", "w_id12", C8,
                     "b_u5w")

            fbuf = sup_p.tile([3, SUP * STRIDE], F32, tag="fbuf")
            for ch in range(SUP):
                cs = slice(ch * 128, (ch + 1) * 128)
                tp = ps.tile([128, 64], BF, tag="zB")
                nc.tensor.transpose(tp[:, 0:24], h1[:, cs], W["eye24"][:])
                T1 = work.tile([128, 24], BF, tag="T1")
                nc.vector.tensor_scalar(T1[:], tp[:, 0:24], 0.0, None, OP.add)
                oh = work.tile([128, 8 * WIN], BF, tag="oh")
                nc.vector.scalar_tensor_tensor(
                    oh[:].rearrange("p (a b) -> p a b", b=WIN),
                    iotaE3, 1.0,
                    rle[:, ch * 8:(ch + 1) * 8].broadcast_to([128, 8, WIN]),
                    OP.mult, OP.is_equal)
                gps = ps.tile([3, WIN], F32, tag="zA")
                for j in range(8):
                    nc.tensor.matmul(gps[:], T1[:, 3 * j:3 * j + 3],
                                     oh[:, j * WIN:(j + 1) * WIN],
                                     start=(j == 0), stop=(j == 7))
                Rnew = work.tile([3, WIN], F32, tag="R0")
                nc.vector.tensor_tensor(Rnew[:, 0:OV], gps[:, 0:OV],
                                        Rprev[:, STRIDE:WIN], OP.add)
                nc.vector.tensor_scalar(Rnew[:, OV:WIN], gps[:, OV:WIN],
                                        0.0, None, OP.add)
                nc.vector.tensor_scalar(fbuf[:, ch * STRIDE:(ch + 1) * STRIDE],
                                        Rprev[:, 0:STRIDE], 0.0, None, OP.add)
                Rprev = Rnew
            nc.sync.dma_start(
                partials_d[:, sup * SUP * STRIDE:(sup + 1) * SUP * STRIDE],
                fbuf[:])

        nc.sync.dma_start(partials_d[:, TCH * STRIDE:TCH * STRIDE + WIN],
                          Rprev[:])
        tail0 = TCH * STRIDE + WIN
        if tail0 < NP:
            zbuf = pers.tile([3, 1024], F32, tag="zbuf")
            nc.vector.memset(zbuf[:], 0.0)
            for o in range(tail0, NP, 1024):
                w_ = min(1024, NP - o)
                nc.sync.dma_start(partials_d[:, o:o + w_], zbuf[:, 0:w_])

        # ----------------- node phase ---------------------------------------
        pn8 = pers.tile([24, NB8], F32, tag="pn8")
        for j in range(8):
            nc.sync.dma_start(pn8[3 * j:3 * j + 3, :],
                              partials_d[:, j * NB8:(j + 1) * NB8])

        c2 = pers.tile([96, NB8], BF, tag="c2")
        h2e = pers.tile([128, NB8], BF, tag="h2e")
        s1n = pers.tile([128, NB8], F32, tag="s1n")

        for o in range(0, NB8, CH):
            sl = slice(o, o + CH)
            zn1 = ps.tile([128, CH], F32, tag="zA")
            mm_acc(zn1, [("w_n1", pn8[:, sl])], CH)
            tn, qn = selu_stage(zn1, CH, "b_n1", True)
            nc.vector.tensor_tensor(s1n[:, sl], tn[:], qn[:], OP.add)

        def wr_c2(zz, sl):
            nc.vector.tensor_scalar(c2[:, sl], zz[:], 0.0, None, OP.add)

        def wr_h2e(zz, sl):
            nc.vector.tensor_scalar(h2e[:, sl], zz[:],
                                    W["b_nhv"][0:128, :], None, OP.add)

        perm_chain(s1n, NB8, "n",
                   [(96, "w_nc2_l", "w_nc2_h", wr_c2),
                    (128, "w_nh_l", "w_nh_h", wr_h2e)])

        nbuf = pers.tile([24, NB8], F32, tag="nbuf")
        nc.sync.dma_start(nbuf[:], dram["n0p"][:])
        h2n = pers.tile([24, NB8], BF, tag="h2n")
        upd_loop(nbuf, c2, h2n, "b_nz1", "b_m", "w_m", "w_id12", NB8,
                 "b_m5w")

        # graph segment sums
        gn_acc = pers.tile([3, G], F32, tag="gn_acc")
        nc.vector.memset(gn_acc[:], 0.0)
        ge_acc = pers.tile([16, G], F32, tag="ge_acc")
        nc.vector.memset(ge_acc[:], 0.0)
        iotaG3 = W["iotaG"][:].rearrange("p (a b) -> p a b", b=G)

        for ch in range(NP // 1024):
            cs = slice(ch * 128, (ch + 1) * 128)
            oh = work.tile([128, 8 * G], BF, tag="oh")
            nc.vector.scalar_tensor_tensor(
                oh[:, 0:8 * G].rearrange("p (a b) -> p a b", b=G), iotaG3, 1.0,
                rlocN8s[:, ch * 8:(ch + 1) * 8].broadcast_to([128, 8, G]),
                OP.mult, OP.is_equal)
            tp = ps.tile([128, 64], BF, tag="zB")
            nc.tensor.transpose(tp[:, 0:24], h2n[:, cs], W["eye24"][:])
            T1 = work.tile([128, 24], BF, tag="T1")
            nc.vector.tensor_scalar(T1[:], tp[:, 0:24], 0.0, None, OP.add)
            gq = ps.tile([3, G], F32, tag="zA")
            for j in range(8):
                nc.tensor.matmul(gq[:], T1[:, 3 * j:3 * j + 3],
                                 oh[:, j * G:(j + 1) * G],
                                 start=(j == 0), stop=(j == 7))
            nc.vector.tensor_tensor(gn_acc[:], gq[:], gn_acc[:], OP.add)

            tpe = ps.tile([128, 128], BF, tag="zB")
            nc.tensor.transpose(tpe[:], h2e[:, cs], W["eye128"][:])
            T1e = work.tile([128, 128], BF, tag="T14")
            nc.vector.tensor_scalar(T1e[:], tpe[:], 0.0, None, OP.add)
            ge = ps.tile([16, G], F32, tag="zA")
            for j in range(8):
                nc.tensor.matmul(ge[:], T1e[:, 16 * j:16 * j + 16],
                                 oh[:, j * G:(j + 1) * G],
                                 start=(j == 0), stop=(j == 7))
            nc.vector.tensor_tensor(ge_acc[:], ge[:], ge_acc[:], OP.add)

        # AllReduce + predict
        bounce_in = dr.tile([19, G], F32, tag="bin")
        bounce_out = dr.tile([19, G], F32, tag="bout")
        nc.gpsimd.dma_start(bounce_in[0:3, :], gn_acc[:])
        nc.gpsimd.dma_start(bounce_in[3:19, :], ge_acc[:])
        nc.gpsimd.collective_compute(
            "AllReduce", mybir.AluOpType.add, replica_groups=[list(range(8))],
            ins=[bounce_in.opt()], outs=[bounce_out.opt()])
        gr_n = pers.tile([3, G], F32, tag="gr_n")
        nc.sync.dma_start(gr_n[:], bounce_out[0:3, :])
        gr_e = pers.tile([16, G], F32, tag="gr_e")
        nc.sync.dma_start(gr_e[:], bounce_out[3:19, :])

        z1 = ps.tile([12, G], F32, tag="zA")
        nc.tensor.matmul(z1[:], W["w_q1n"][:], gr_n[:], start=True, stop=False)
        nc.tensor.matmul(z1[:], W["w_q1e"][:], gr_e[:], start=False, stop=True)
        t, q = selu_stage(z1, G, "b_q1", True)
        for li, rows in ((2, 12), (3, 12), (4, 8)):
            zi = ps.tile([12, G], F32, tag="zB" if li % 2 == 0 else "zA")
            nc.tensor.matmul(zi[0:rows, :], W[f"w_q{li}"][:], t[:],
                             start=True, stop=False)
            nc.tensor.matmul(zi[0:rows, :], W[f"w_q{li}"][:], q[:],
                             start=False, stop=True)
            t, q = selu_stage(zi[0:rows, :], G, f"b_q{li}", True)
        z5 = ps.tile([9, G], F32, tag="zB")
        nc.tensor.matmul(z5[:], W["w_q5"][:], t[:], start=True, stop=False)
        nc.tensor.matmul(z5[:], W["w_q5"][:], q[:], start=False, stop=True)
        outt = pers.tile([12, G], F32, tag="outt")
        nc.vector.memset(outt[:], 0.0)
        nc.vector.tensor_scalar(outt[0:9, :], z5[:], W["b_q5w"][0:9, :],
                                None, OP.add)
        nc.sync.dma_start(out_ext[:], outt[:])

    nc.compile()
    return nc
